# revision 8
# baseline (speedup 1.0000x reference)
"""Trainium2 Bass kernel for a 2-layer LSTM + Dense head.

Model (per reference):
  L1: LSTM(H1=32, tanh),  L2: LSTM(H2=16, relu), Dense(12) on last h2.
  x: [512, 512, 64] f32.

Strategy: pure data parallelism, batch 512 -> 64 per core over 8 cores.
Per core, both layers are merged into shared engine ops by stacking their
hidden rows on partitions: rows [h1(0:32) | h2(32:48) | ones(48)].
Gates are laid out along the free dim in blocks [g|i|f|o] x 64(batch), so the
whole cell update for BOTH layers is:
  - 12 tiny matmuls (4 input-proj off critical path, 8 recurrent on-chain)
  - sigmoid over [g|i|f] blocks in one ACT op (L1 g-cols pre-doubled so that
    tanh(g) = 2*sigmoid(2g)-1 is recovered with one cheap DVE op)
  - one fused TT mul producing [i*g | f*c], one TT add -> c
  - tanh(c1) on ACT, relu(c2)/relu(g2) on DVE, one TT mul -> h
x is transposed to [F, batch] per step via off-critical-path PE transposes.
"""

import sys

import numpy as np

if "/opt/trn_rl_repo" not in sys.path:
    sys.path.insert(0, "/opt/trn_rl_repo")

B_FULL = 512
T_FULL = 512
F = 64
H1, H2, OUT = 32, 16, 12
N_CORES = 8
B = B_FULL // N_CORES  # 64 batch per core

L1R0, L1R1 = 0, H1          # L1 rows 0:32
L2R0, L2R1 = H1, H1 + H2    # L2 rows 32:48
NR = H1 + H2                # 48
ONESROW = NR                # row 48 = ones

_NC_CACHE = {}


def build_nc(T=T_FULL, unroll_feed=True):
    import concourse.mybir as mybir
    from concourse import bacc
    from concourse.masks import make_identity
    from concourse.tile import TileContext

    fp32 = mybir.dt.float32
    Sig = mybir.ActivationFunctionType.Sigmoid
    Tanh = mybir.ActivationFunctionType.Tanh
    mult = mybir.AluOpType.mult
    add = mybir.AluOpType.add

    CT = 32 if T >= 32 else T   # x DMA chunk (timesteps)
    LA = 16 if T >= 32 else T   # transpose lookahead
    CPY = 8 if T >= 8 else T    # timesteps per PSUM->SBUF xT copy
    XT_RING = 32 if T >= 32 else T  # xT ring slots

    nc = bacc.Bacc(None, target_bir_lowering=False)

    x_d = nc.dram_tensor("x", [B, T, F], fp32, kind="ExternalInput")
    wA_d = nc.dram_tensor("wA", [F + 1, 4 * NR], fp32, kind="ExternalInput")
    wB_d = nc.dram_tensor("wB", [NR + 1, 4 * NR], fp32, kind="ExternalInput")
    wD_d = nc.dram_tensor("wD", [NR + 1, OUT], fp32, kind="ExternalInput")
    ri_d = nc.dram_tensor("rinit", [NR + 1, B], fp32, kind="ExternalInput")
    out_d = nc.dram_tensor("out", [OUT, B], fp32, kind="ExternalOutput")

    with TileContext(nc) as tc:
        with (
            tc.tile_pool(name="singles", bufs=1) as sp,
            tc.tile_pool(name="xraw", bufs=2) as xrp,
            tc.tile_pool(name="psum_z", bufs=4, space="PSUM") as pz,
            tc.tile_pool(name="psum_t", bufs=2, space="PSUM") as pt,
            tc.tile_pool(name="psum_o", bufs=1, space="PSUM") as po,
        ):
            wA = sp.tile([F + 1, 4 * NR], fp32)
            wB = sp.tile([NR + 1, 4 * NR], fp32)
            wD = sp.tile([NR + 1, OUT], fp32)
            nc.sync.dma_start(wA[:], wA_d[:])
            nc.sync.dma_start(wB[:], wB_d[:])
            nc.sync.dma_start(wD[:], wD_d[:])

            ident = sp.tile([64, 64], fp32)
            make_identity(nc, ident[:])

            # recurrent state [h1(0:32); h2(32:48); ones(48)] x batch, x2 (ping/pong)
            rhsA = sp.tile([NR + 1, B], fp32)
            rhsB = sp.tile([NR + 1, B], fp32)
            rhs = [rhsA, rhsB]
            for r in rhs:  # zeros + ones row 48 (compute ops can't start at p48)
                nc.sync.dma_start(r[:], ri_d[:])

            GC = sp.tile([NR, 2 * B], fp32)  # cols [g' | c]
            nc.gpsimd.memset(GC[:], 0.0)
            S = sp.tile([NR, 4 * B], fp32)   # sigma(z) blocks [g|i|f|o]
            M = sp.tile([NR, 2 * B], fp32)   # [i*g | f*c]
            TH = sp.tile([NR, B], fp32)      # [tanh(c1); relu(c2)]

            xT = sp.tile([F + 1, XT_RING * B], fp32)  # transposed x ring + ones row
            nc.gpsimd.memset(xT[F : F + 1, :], 1.0)

            state = {"xraw": None, "psumT": None}

            def feed(k):
                t = k + LA
                if t >= T or t < 0:
                    return
                if t % CT == 0:
                    state["xraw"] = xrp.tile([B, CT * F], fp32, tag="xraw", name="xraw")
                    nc.sync.dma_start(state["xraw"][:], x_d[:, t : t + CT, :])
                if t % CPY == 0:
                    state["psumT"] = pt.tile([F, CPY * B], fp32, tag="psumT", name="psumT")
                j = t % CT
                nc.tensor.transpose(
                    state["psumT"][:, (t % CPY) * B : (t % CPY + 1) * B],
                    state["xraw"][:, j * F : (j + 1) * F],
                    ident[:],
                )
                if t % CPY == CPY - 1:
                    base = (t - (CPY - 1)) % XT_RING
                    nc.scalar.copy(
                        xT[0:F, base * B : (base + CPY) * B], state["psumT"][:]
                    )

            for k in range(-LA, 0):
                feed(k)

            for k in range(T + 1):
                feed(k)
                r_cur = rhs[k % 2]
                r_nxt = rhs[(k + 1) % 2]
                last = k == T
                # active rows for the merged elementwise ops:
                # k=0 -> L1 only (L2 state must stay zero until its first
                # real step at k=1), k=T -> L2 only (epilogue), else both.
                if k == 0:
                    ra, rb = 0, H1
                elif last:
                    ra, rb = L2R0, L2R1
                else:
                    ra, rb = 0, NR
                z = pz.tile([NR, 4 * B], fp32, tag="z", name="z")

                # PSUM zero regions are 2KB (the whole bank row), so the
                # first matmul starts the group and the last one stops it.
                # mmA (input proj, cols 32:48 zero-padded) opens rows 0:48 off
                # the critical path; the merged recurrent matmul does
                # [U1;0 | W2;U2;b2]^T [h1;h2;ones] for one gate in ONE op.
                if not last:
                    rk = k % XT_RING
                    for j in range(4):
                        nc.tensor.matmul(
                            z[0:NR, j * B : (j + 1) * B],
                            wA[:, j * NR : (j + 1) * NR],
                            xT[:, rk * B : (rk + 1) * B],
                            start=(j == 0),
                            stop=False,
                        )
                for j in range(4):
                    nc.tensor.matmul(
                        z[0:NR, j * B : (j + 1) * B],
                        wB[:, j * NR : (j + 1) * NR],
                        r_cur[0 : NR + 1, :],
                        start=(j == 0 and last),
                        stop=(j == 3),
                    )

                zl2 = k > 0              # L2 z rows valid this iter
                if zl2:  # relu(z_g2) straight from PSUM, early on DVE
                    nc.vector.tensor_scalar_max(
                        GC[L2R0:L2R1, 0:B], z[L2R0:L2R1, 0:B], 0.0
                    )
                if not last:  # tanh(g1) straight from PSUM (same ACT table set)
                    nc.scalar.activation(GC[L1R0:L1R1, 0:B], z[L1R0:L1R1, 0:B], Tanh)
                # sigmoid over [i|f] blocks (one op), then the o block
                nc.scalar.activation(S[ra:rb, B : 3 * B], z[ra:rb, B : 3 * B], Sig)
                nc.scalar.activation(
                    S[ra:rb, 3 * B : 4 * B], z[ra:rb, 3 * B : 4 * B], Sig
                )
                # c update: [i*g | f*c] then add
                nc.vector.tensor_mul(
                    M[ra:rb, :], S[ra:rb, B : 3 * B], GC[ra:rb, :]
                )
                nc.vector.tensor_add(
                    GC[ra:rb, B : 2 * B], M[ra:rb, 0:B], M[ra:rb, B : 2 * B]
                )
                if not last:
                    nc.scalar.activation(
                        TH[L1R0:L1R1, :], GC[L1R0:L1R1, B : 2 * B], Tanh
                    )
                if zl2:
                    nc.vector.tensor_scalar_max(
                        TH[L2R0:L2R1, :], GC[L2R0:L2R1, B : 2 * B], 0.0
                    )
                # h = act(c) * sigma(o) -> next-step rhs
                nc.vector.tensor_mul(
                    r_nxt[ra:rb, :], TH[ra:rb, :], S[ra:rb, 3 * B : 4 * B]
                )

            # dense head: [0(h1); Wd(h2); bd]^T [h1; h2; ones]
            r_fin = rhs[(T + 1) % 2]
            opsum = po.tile([OUT, B], fp32, tag="o", name="opsum")
            nc.tensor.matmul(
                opsum[:], wD[:], r_fin[0 : NR + 1, :], start=True, stop=True
            )
            osb = sp.tile([OUT, B], fp32)
            nc.scalar.copy(osb[:], opsum[:])
            nc.sync.dma_start(out_d[:], osb[:])

    nc.compile()
    return nc


def _get_nc(T=T_FULL):
    if T not in _NC_CACHE:
        _NC_CACHE[T] = build_nc(T)
    return _NC_CACHE[T]


def prep_weights(W1, U1, b1, W2, U2, b2, Wd, bd):
    """Pack weights into the 4 lhsT tensors (gate blocks [g,i,f,o])."""

    def gates(w, H):
        w = np.asarray(w, np.float32)
        i, f, g, o = (w[..., k * H : (k + 1) * H] for k in range(4))
        return [g, i, f, o]  # block order

    W1g, b1g = gates(W1, H1), gates(b1, H1)
    W2g, U1g, U2g, b2g = gates(W2, H2), gates(U1, H1), gates(U2, H2), gates(b2, H2)
    # wA[j]: [65, 48] = [[W1_j; b1_j] | zeros]
    wA = np.concatenate(
        [
            np.concatenate(
                [
                    np.concatenate([W1g[j], b1g[j][None, :]], axis=0),
                    np.zeros((F + 1, H2), np.float32),
                ],
                axis=1,
            )
            for j in range(4)
        ],
        axis=1,
    ).astype(np.float32)
    # wB[j]: [49, 48] = [[U1_j; 0] | [W2_j; U2_j; b2_j]]
    wB = np.concatenate(
        [
            np.concatenate(
                [
                    np.concatenate(
                        [U1g[j], np.zeros((H2 + 1, H1), np.float32)], axis=0
                    ),
                    np.concatenate(
                        [W2g[j], U2g[j], b2g[j][None, :]], axis=0
                    ),
                ],
                axis=1,
            )
            for j in range(4)
        ],
        axis=1,
    ).astype(np.float32)
    wD = np.concatenate(
        [
            np.zeros((H1, OUT), np.float32),
            np.asarray(Wd, np.float32),
            np.asarray(bd, np.float32)[None, :],
        ],
        axis=0,
    ).astype(np.float32)
    return wA, wB, wD


def run_cores(nc, x, weights, T, trace=False):
    from concourse.bass_utils import run_bass_kernel_spmd

    weights = dict(weights)
    rinit = np.zeros((NR + 1, B), np.float32)
    rinit[NR, :] = 1.0
    weights["rinit"] = rinit
    x = np.ascontiguousarray(np.asarray(x, np.float32))
    in_maps = [
        dict(x=np.ascontiguousarray(x[c * B : (c + 1) * B, :T]), **weights)
        for c in range(N_CORES)
    ]
    res = run_bass_kernel_spmd(nc, in_maps, core_ids=list(range(N_CORES)), trace=trace)
    out = np.concatenate([r["out"].T for r in res.results], axis=0)
    return out.astype(np.float32), res


def kernel(x, W1, U1, b1, W2, U2, b2, Wd, bd):
    wA, wB, wD = prep_weights(W1, U1, b1, W2, U2, b2, Wd, bd)
    nc = _get_nc(T_FULL)
    out, _ = run_cores(nc, x, dict(wA=wA, wB=wB, wD=wD), T_FULL)
    return out
